# revision 19
# baseline (speedup 1.0000x reference)
"""FNS spectral network kernel v7 for 8x TRN2 NeuronCores (1 sample/core).

Math per sample b (validated vs reference in fp64 numpy, rel err ~3e-7):
    rh = (-Gi) @ r @ Gi.T ; x = conv1..conv3 -> *theta -> conv4..conv6 ;
    e  = H @ x @ H.T

v7 = v2 device mechanics (proven on HW) + scheduling fixes:
  - const loads reordered: front-critical (r16, g's, t1) on sync; all bulk
    tensors (t2..t6 via tcat, hcat, thet) on the gpsimd SWDGE ring so the
    HWDGE queues stay free for the x1 scatter / halo exchange.
  - thet fully SBUF-resident (prefetched at t=0) -- no conv3 DMA stalls.
  - xoc scatter DMAs interleaved into the conv6 loop (hidden behind
    compute) instead of one serial burst afterwards.
  - fp16 outputs; output DMAs split sync/scalar and interleaved with the
    e-stage so the tail is short.
"""

import os

import numpy as np

import concourse.bacc as bacc
import concourse.mybir as mybir
from concourse.ap import AP as APc
from concourse.bass_utils import run_bass_kernel_spmd
from concourse.tile import TileContext

F16 = mybir.dt.float16
F32 = mybir.dt.float32

B = 8
N1 = 255
CROP = 257
CH = 8
NBLK = 43
BSTR = 260
XW = NBLK * BSTR
CHUNKS = [(0, 11), (11, 11), (22, 11), (33, 10)]   # (start, count)
THW = 2 * CROP                                     # theta cols per block
TKEYS = ["t2", "t2z", "t2b", "t3", "t3z", "t3b", "t3s", "t3sz", "t3sb",
         "t4", "t4z", "t4b", "t5", "t5z", "t5b"]
TOFF = {k: i * 384 for i, k in enumerate(TKEYS)}

LAST_EXEC_TIME_NS = None


# ----------------------------------------------------------------------------
# Host-side prep
# ----------------------------------------------------------------------------

def _host_consts():
    j = np.arange(CROP)[:, None]
    n = np.arange(N1)[None, :]
    Gi = (np.sin(np.pi * (j - 128) * (n + 1) / 256.0) / 256.0).astype(np.float32)
    k = np.arange(N1)[:, None]
    jj = np.arange(CROP)[None, :]
    H = np.exp(-2j * np.pi * k * (jj - 127.0) / 513.0)
    g1t = np.ascontiguousarray((-Gi).T.astype(np.float16))   # [255,257]
    g2t = np.ascontiguousarray(Gi.T.astype(np.float16))      # [255,257]
    hrt = np.ascontiguousarray(H.real.T.astype(np.float16))  # [257,255]
    hit = np.ascontiguousarray(H.imag.T.astype(np.float16))
    hnit = np.ascontiguousarray((-H.imag).T.astype(np.float16))
    return {
        "gcat": np.ascontiguousarray(np.concatenate([g1t, g2t], axis=1)),
        "hcat": np.ascontiguousarray(np.concatenate([hrt, hit, hnit], axis=1)),
    }


def _expand_w(wre, wim):
    Co, Ci = wre.shape[0], wre.shape[1]
    W = np.zeros((2 * Co, 2 * Ci, 3, 3), np.float32)
    W[:Co, :Ci] = wre
    W[:Co, Ci:] = -wim
    W[Co:, :Ci] = wim
    W[Co:, Ci:] = wre
    return W


def _wT(wre, wim):
    return (np.swapaxes(np.swapaxes(wre, 0, 1), -2, -1),
            -np.swapaxes(np.swapaxes(wim, 0, 1), -2, -1))


def _row_std(p):
    if p < 96:
        return 1 + p // 16, p % 16
    if p < 112:
        return 0, p - 96
    return 7, p - 112


def _col_std_dup(m):
    if m < 96:
        return m // 16, m % 16
    if m < 112:
        return 5, m - 96
    return 0, m - 112


def _col_c6(m):
    return m % 6, m // 6


def _build_T(Wexp, rowmap, colmap, K, M, zero_hi=False, zero_lo=False):
    T = np.zeros((K, 3 * M), np.float32)
    Cin2 = Wexp.shape[1]
    for p in range(K):
        il, cp = rowmap(p)
        if cp >= Cin2:
            continue
        if zero_hi and il >= 6:
            continue
        if zero_lo and il == 0:
            continue
        for dj in range(3):
            for m in range(M):
                inn, op = colmap(m)
                di = il - inn
                if 0 <= di <= 2:
                    T[p, dj * M + m] = Wexp[op, cp, di, dj]
    return T.astype(np.float16)


def _host_prep_sample(bidx, inputs, consts):
    s = {}
    s["r16"] = np.ascontiguousarray(inputs["r"][bidx, 0].astype(np.float16))
    s.update(consts)

    w1 = (inputs["w1_re"][bidx], inputs["w1_im"][bidx])
    w2 = (inputs["w2_re"][bidx], inputs["w2_im"][bidx])
    w3 = (inputs["w3_re"][bidx], inputs["w3_im"][bidx])

    W1r = _expand_w(*w1)[:, 0:1]
    W2 = _expand_w(*w2)
    W3 = _expand_w(*w3)
    W3s = np.concatenate([W3[CH:], W3[:CH]], axis=0)
    W4 = _expand_w(*_wT(*w3))
    W5 = _expand_w(*_wT(*w2))
    W6 = _expand_w(*_wT(*w1))

    def row_x1(p):
        return p, 0

    s["t1cat"] = np.ascontiguousarray(np.concatenate([
        _build_T(W1r, row_x1, _col_std_dup, 8, 128),
        _build_T(W1r, row_x1, _col_std_dup, 8, 128, zero_hi=True)], axis=1))

    tm = {}
    for key, W in (("t2", W2), ("t3", W3), ("t3s", W3s), ("t4", W4), ("t5", W5)):
        tm[key] = _build_T(W, _row_std, _col_std_dup, 128, 128)
        tm[key + "z"] = _build_T(W, _row_std, _col_std_dup, 128, 128, zero_lo=True)
        tm[key + "b"] = _build_T(W, _row_std, _col_std_dup, 128, 128, zero_hi=True)
    s["tcat"] = np.ascontiguousarray(
        np.concatenate([tm[k] for k in TKEYS], axis=1))
    s["t6cat"] = np.ascontiguousarray(np.concatenate([
        _build_T(W6, _row_std, _col_c6, 128, 12),
        _build_T(W6, _row_std, _col_c6, 128, 12, zero_lo=True),
        _build_T(W6, _row_std, _col_c6, 128, 12, zero_hi=True)], axis=1))

    # theta pack [128, NBLK*514]; sign baked: col0 block = +tr, col1 = -/+ti
    tr = inputs["theta_re"][bidx]
    ti = inputs["theta_im"][bidx]
    th = np.zeros((128, NBLK * THW), np.float16)
    for b in range(NBLK):
        base = b * THW
        ninn = 6 if b < NBLK - 1 else 5
        for p in range(128):
            if p < 96:
                inn, op = p // 16, p % 16
            elif p < 112:
                inn, op = 5, p - 96
            else:
                inn, op = 0, p - 112
            if inn >= ninn:
                continue
            row = 6 * b + inn
            ch = op % 8
            th[p, base:base + CROP] = tr[ch, row]
            th[p, base + CROP:base + THW] = (-ti[ch, row]) if op < 8 else ti[ch, row]
    s["thet"] = th
    return s


# ----------------------------------------------------------------------------
# Device program
# ----------------------------------------------------------------------------

def _build_nc():
    nc = bacc.Bacc(None, target_bir_lowering=False, debug=False)

    dp = {}
    decls = [("r16", [N1, N1]), ("gcat", [N1, 2 * CROP]),
             ("hcat", [CROP, 3 * N1]), ("t1cat", [8, 768]),
             ("tcat", [128, 15 * 384]), ("t6cat", [128, 108]),
             ("thet", [128, NBLK * THW])]
    for name, shape in decls:
        dp[name] = nc.declare_dram_parameter(name, list(shape), F16,
                                             isOutput=False)
    ere = nc.declare_dram_parameter("ere", [N1, N1], F16, isOutput=True)
    eim = nc.declare_dram_parameter("eim", [N1, N1], F16, isOutput=True)

    with TileContext(nc) as tc:
        with (
            tc.tile_pool(name="const", bufs=1) as pc,
            tc.tile_pool(name="xbuf", bufs=1) as px,
            tc.tile_pool(name="work", bufs=1) as pw,
            tc.tile_pool(name="wk2", bufs=3) as pw2,
            tc.tile_pool(name="psum", bufs=8, space="PSUM") as pp,
            tc.tile_pool(name="dscr", bufs=1, space="DRAM") as pd,
        ):
            scr1 = pd.tile([261, CROP], F16, name="scr1", tag="scr1")
            zrow = pc.tile([1, CROP], F16, name="zrow", tag="zrow")
            nc.vector.memset(zrow[:, :], 0.0)
            # ---------------- constant loads ----------------
            # front-critical on sync (HWDGE); bulk on gpsimd (SWDGE)
            r_sb = [pc.tile([128, N1], F16, name="r0", tag="r0"),
                    pc.tile([127, N1], F16, name="r1", tag="r1")]
            nc.sync.dma_start(r_sb[0][:, :], dp["r16"][0:128, :])
            nc.sync.dma_start(r_sb[1][:, :], dp["r16"][128:255, :])
            g_sb = [pc.tile([128, 2 * CROP], F16, name="g0", tag="g0"),
                    pc.tile([127, 2 * CROP], F16, name="g1", tag="g1")]
            nc.sync.dma_start(g_sb[0][:, :], dp["gcat"][0:128, :])
            nc.sync.dma_start(g_sb[1][:, :], dp["gcat"][128:255, :])
            t1c = pc.tile([8, 768], F16, name="t1c", tag="t1c")
            nc.sync.dma_start(t1c[:, :], dp["t1cat"][:, :])

            # guard rows for the x1 gather (rows -1 / 257+ of rh read as 0)
            nc.scalar.dma_start(scr1[0:1, :], zrow[0:1, :])
            nc.scalar.dma_start(scr1[258:259, :], zrow[0:1, :])
            nc.scalar.dma_start(scr1[259:260, :], zrow[0:1, :])
            # bulk loads wait for the front-critical bytes (HBM bw is shared)
            nc.gpsimd.dma_start(scr1[260:261, :], g_sb[1][0:1, 0:CROP])
            tcat = pc.tile([128, 15 * 384], F16, name="tcat", tag="tcat")
            nc.gpsimd.dma_start(tcat[:, :], dp["tcat"][:, :])
            t6c = pc.tile([128, 108], F16, name="t6c", tag="t6c")
            nc.gpsimd.dma_start(t6c[:, :], dp["t6cat"][:, :])
            thet = pc.tile([128, NBLK * THW], F16, name="thet", tag="thet")
            half = (NBLK // 2) * THW
            nc.gpsimd.dma_start(thet[:, 0:half], dp["thet"][:, 0:half])
            nc.gpsimd.dma_start(thet[:, half:], dp["thet"][:, half:])
            h_sb = [pc.tile([128, 3 * N1], F16, name="h0", tag="h0"),
                    pc.tile([128, 3 * N1], F16, name="h1", tag="h1"),
                    pc.tile([1, 3 * N1], F16, name="h2", tag="h2")]
            nc.gpsimd.dma_start(h_sb[0][:, :], dp["hcat"][0:128, :])
            nc.gpsimd.dma_start(h_sb[1][:, :], dp["hcat"][128:256, :])
            nc.gpsimd.dma_start(h_sb[2][:, :], dp["hcat"][256:257, :])

            def hslice(k2, which):
                return h_sb[k2][:, which * N1:(which + 1) * N1]

            def tvar(key, b):
                if b == 0 and key + "z" in TOFF:
                    key = key + "z"
                elif b == NBLK - 1 and key + "b" in TOFF:
                    key = key + "b"
                return TOFF[key]

            def tls(key, b, dj):
                off = tvar(key, b)
                return tcat[:, off + dj * 128:off + (dj + 1) * 128]

            def t1ls(b, dj):
                off = 384 if b == NBLK - 1 else 0
                return t1c[:, off + dj * 128:off + (dj + 1) * 128]

            # ---------------- big X tiles + pads ----------------
            x1 = px.tile([8, XW], F16, name="x1", tag="x1")
            XA = px.tile([128, XW], F16, name="XA", tag="XA")
            XB = px.tile([128, XW], F16, name="XB", tag="XB")
            XC = px.tile([128, XW], F16, name="XC", tag="XC")
            y6 = px.tile([12, NBLK * CROP], F16, name="y6", tag="y6")

            for X in (x1, XA, XB, XC):
                v = X[:, :].rearrange("p (b c) -> p b c", c=BSTR)
                nc.vector.memset(v[:, :, 0:1], 0.0)
                nc.vector.memset(v[:, :, 258:260], 0.0)
            nc.vector.memset(x1[0:1, 0:BSTR], 0.0)
            nc.vector.memset(x1[0:8, BSTR * 42:], 0.0)
            for X in (XA, XB, XC):
                nc.vector.memset(X[96:112, 0:BSTR], 0.0)
                nc.vector.memset(X[96:128, BSTR * 42:], 0.0)

            # ---------------- front transform ----------------
            vt_sb = [pw.tile([128, CROP], F16, name="vt0", tag="vt0"),
                     pw.tile([127, CROP], F16, name="vt1", tag="vt1")]
            for m, (m0, mm) in enumerate(((0, 128), (128, 127))):
                ps = pp.tile([128, CROP], F32, name="ps", tag="ps", bufs=3)
                for k2 in range(2):
                    nc.tensor.matmul(
                        ps[0:mm, :], lhsT=r_sb[k2][:, m0:m0 + mm],
                        rhs=g_sb[k2][:, 0:CROP], start=(k2 == 0), stop=(k2 == 1))
                nc.scalar.copy(vt_sb[m][:, :], ps[0:mm, :])

            rh_sb = [pw.tile([128, CROP], F16, name="rh0", tag="rh0"),
                     pw.tile([128, CROP], F16, name="rh1", tag="rh1"),
                     pw.tile([1, CROP], F16, name="rh2", tag="rh2")]
            for m, (m0, mm) in enumerate(((0, 128), (128, 128), (256, 1))):
                ps = pp.tile([128, CROP], F32, name="ps", tag="ps", bufs=3)
                for k2 in range(2):
                    nc.tensor.matmul(
                        ps[0:mm, :], lhsT=vt_sb[k2][:, m0:m0 + mm],
                        rhs=g_sb[k2][:, CROP:2 * CROP],
                        start=(k2 == 0), stop=(k2 == 1))
                nc.vector.tensor_copy(rh_sb[m][:, :], ps[0:mm, :])
                # store rh rows r -> scr1 row r+1 as soon as evicted
                (nc.sync, nc.scalar, nc.sync)[m].dma_start(
                    scr1[m0 + 1:m0 + 1 + mm, :], rh_sb[m][:, :])

            # x1 gather: x1[il, block b cols] = scr1[6b+il] (= rh[6b-1+il]);
            # one strided DRAM-source DMA replaces ~45 small scatters.
            x1v = x1[:, :].rearrange("p (b c) -> p b c", c=BSTR)
            nc.sync.dma_start(
                x1v[0:8, 0:NBLK, 1:258],
                APc(scr1[:, :].tensor, 0,
                    [[CROP, 8], [6 * CROP, NBLK], [1, CROP]]))

            # ---------------- conv machinery ----------------
            S_W = 11 * CROP

            def strips_stage(XO, cstart, cnt, Svar):
                xv = XO[:, :].rearrange("p (b c) -> p b c", c=BSTR)
                sv = Svar[:, :].rearrange("p (b c) -> p b c", c=CROP)
                nc.sync.dma_start(sv[:, 0:cnt, :],
                                  xv[96:128, cstart:cstart + cnt, 1:258])

            def strips_fill_main(XO, cstart, cnt, Svar, eng_r):
                xv = XO[:, :].rearrange("p (b c) -> p b c", c=BSTR)
                sv = Svar[:, :].rearrange("p (b c) -> p b c", c=CROP)
                nb2 = cnt - 1
                if nb2 > 0:
                    eng_r.dma_start(
                        xv[96:112, cstart + 1:cstart + 1 + nb2, 1:258],
                        sv[0:16, 0:nb2, :])
                o = 1 if cstart == 0 else 0
                nb3 = cnt - o
                if nb3 > 0:
                    nc.scalar.dma_start(
                        xv[112:128, cstart + o - 1:cstart + o - 1 + nb3, 1:258],
                        sv[16:32, o:o + nb3, :])

            def strips_fill_cross(XO, cstart, cnt, Svar, eng_r):
                if cstart + cnt >= NBLK:
                    return
                xv = XO[:, :].rearrange("p (b c) -> p b c", c=BSTR)
                sv = Svar[:, :].rearrange("p (b c) -> p b c", c=CROP)
                eng_r.dma_start(
                    xv[96:112, cstart + cnt:cstart + cnt + 1, 1:258],
                    sv[0:16, cnt - 1:cnt, :])

            def conv_layer(tsel, XI, XO, kin, eng_r):
                prevS = None
                for ci, (cstart, cnt) in enumerate(CHUNKS):
                    for b in range(cstart, cstart + cnt):
                        ps = pp.tile([128, CROP], F32, name="ps", tag="ps",
                                     bufs=3)
                        for dj in range(3):
                            nc.tensor.matmul(
                                ps[:, :], lhsT=tsel(b, dj),
                                rhs=XI[0:kin, BSTR * b + dj:BSTR * b + dj + CROP],
                                start=(dj == 0), stop=(dj == 2))
                        dst = XO[:, BSTR * b + 1:BSTR * b + 1 + CROP]
                        if b % 2 == 0:
                            nc.vector.tensor_copy(dst, ps[:, :])
                        else:
                            nc.scalar.copy(dst, ps[:, :])
                    Svar = pw2.tile([32, S_W], F16, name="S", tag="S", bufs=3)
                    strips_stage(XO, cstart, cnt, Svar)
                    if prevS is not None:
                        strips_fill_cross(XO, *prevS, eng_r)
                    strips_fill_main(XO, cstart, cnt, Svar, eng_r)
                    prevS = (cstart, cnt, Svar)

            conv_layer(t1ls, x1, XA, 8, nc.sync)
            conv_layer(lambda b, dj: tls("t2", b, dj), XA, XB, 128, nc.sync)

            # conv3 + theta: XB -> XC (thet is SBUF-resident; no chunk DMAs)
            prevS = None
            for ci, (cstart, cnt) in enumerate(CHUNKS):
                for b in range(cstart, cstart + cnt):
                    tb = b * THW
                    psA = pp.tile([128, CROP], F32, name="psA", tag="psA",
                                  bufs=2)
                    psB = pp.tile([128, CROP], F32, name="psB", tag="psB",
                                  bufs=2)
                    for dj in range(3):
                        rhs = XB[:, BSTR * b + dj:BSTR * b + dj + CROP]
                        nc.tensor.matmul(
                            psA[:, :], lhsT=tls("t3", b, dj),
                            rhs=rhs, start=(dj == 0), stop=(dj == 2))
                    for dj in range(3):
                        rhs = XB[:, BSTR * b + dj:BSTR * b + dj + CROP]
                        nc.tensor.matmul(
                            psB[:, :], lhsT=tls("t3s", b, dj),
                            rhs=rhs, start=(dj == 0), stop=(dj == 2))
                    u = pw2.tile([128, CROP], F16, name="u", tag="u", bufs=4)
                    v = pw2.tile([128, CROP], F16, name="v", tag="v", bufs=4)
                    nc.vector.tensor_mul(u[:, :], psA[:, :],
                                         thet[:, tb:tb + CROP])
                    nc.vector.tensor_mul(v[:, :], psB[:, :],
                                         thet[:, tb + CROP:tb + THW])
                    nc.gpsimd.tensor_add(
                        XC[:, BSTR * b + 1:BSTR * b + 1 + CROP],
                        u[:, :], v[:, :])
                Svar = pw2.tile([32, S_W], F16, name="S", tag="S", bufs=3)
                strips_stage(XC, cstart, cnt, Svar)
                if prevS is not None:
                    strips_fill_cross(XC, *prevS, nc.sync)
                strips_fill_main(XC, cstart, cnt, Svar, nc.sync)
                prevS = (cstart, cnt, Svar)

            conv_layer(lambda b, dj: tls("t4", b, dj), XC, XA, 128, nc.gpsimd)
            conv_layer(lambda b, dj: tls("t5", b, dj), XA, XB, 128, nc.gpsimd)

            # conv6: XB -> y6, with the xoc scatter interleaved per block
            xoc = [pw.tile([128, 2 * CROP], F16, name="xoc0", tag="xoc0"),
                   pw.tile([128, 2 * CROP], F16, name="xoc1", tag="xoc1"),
                   pw.tile([1, 2 * CROP], F16, name="xoc2", tag="xoc2")]

            def scatter_block(b):
                pieces = []
                ninn = 6 if b < NBLK - 1 else 5
                i0 = 0
                while i0 < ninn:
                    r = 6 * b + i0
                    c = r // 128
                    csz = 128 if c < 2 else 1
                    iend = min(ninn - 1, (c * 128 + csz - 1 - 6 * b))
                    pieces.append((c, i0, iend - i0 + 1))
                    i0 = iend + 1
                for (c, i0, ni) in pieces:
                    p0 = 6 * b + i0 - 128 * c
                    for op in range(2):
                        eng = (nc.sync, nc.scalar, nc.gpsimd)[(2 * b + op) % 3]
                        eng.dma_start(
                            xoc[c][p0:p0 + ni, op * CROP:op * CROP + CROP],
                            y6[6 * op + i0:6 * op + i0 + ni,
                               CROP * b:CROP * (b + 1)])

            for b in range(NBLK):
                off = 36 if b == 0 else (72 if b == NBLK - 1 else 0)
                ps = pp.tile([128, CROP], F32, name="ps", tag="ps", bufs=3)
                for dj in range(3):
                    nc.tensor.matmul(
                        ps[0:12, :], lhsT=t6c[:, off + dj * 12:off + dj * 12 + 12],
                        rhs=XB[:, BSTR * b + dj:BSTR * b + dj + CROP],
                        start=(dj == 0), stop=(dj == 2))
                dst = y6[:, CROP * b:CROP * (b + 1)]
                if b % 2 == 0:
                    nc.vector.tensor_copy(dst, ps[0:12, :])
                else:
                    nc.scalar.copy(dst, ps[0:12, :])
                if b >= 2:
                    scatter_block(b - 2)
            scatter_block(NBLK - 2)
            scatter_block(NBLK - 1)

            # ---------------- back transform ----------------
            at = {}
            for p in ("re", "im"):
                at[p] = [pw.tile([128, N1], F16, name=f"at{p}0", tag=f"at{p}0"),
                         pw.tile([128, N1], F16, name=f"at{p}1", tag=f"at{p}1"),
                         pw.tile([1, N1], F16, name=f"at{p}2", tag=f"at{p}2")]
            for m, (m0, mm) in enumerate(((0, 128), (128, 128), (256, 1))):
                for p, terms in (("re", ((0, 0), (1, 2))),
                                 ("im", ((0, 1), (1, 0)))):
                    ps = pp.tile([128, N1], F32, name="ps", tag="ps", bufs=3)
                    nmm = 0
                    for (xi, hw) in terms:
                        for k2 in range(3):
                            nc.tensor.matmul(
                                ps[0:mm, :],
                                lhsT=xoc[k2][:, xi * CROP + m0:
                                             xi * CROP + m0 + mm],
                                rhs=hslice(k2, hw),
                                start=(nmm == 0), stop=(nmm == 5))
                            nmm += 1
                    nc.scalar.copy(at[p][m][:, :], ps[0:mm, :])

            e_sb = {}
            for p in ("re", "im"):
                e_sb[p] = [pw.tile([128, N1], F16, name=f"e{p}0", tag=f"e{p}0"),
                           pw.tile([127, N1], F16, name=f"e{p}1", tag=f"e{p}1")]
            for m, (m0, mm) in enumerate(((0, 128), (128, 127))):
                for p, terms in (("re", (("re", 0), ("im", 2))),
                                 ("im", (("re", 1), ("im", 0)))):
                    ps = pp.tile([128, N1], F32, name="ps", tag="ps", bufs=3)
                    nmm = 0
                    for (ap_, hw) in terms:
                        for k2 in range(3):
                            nc.tensor.matmul(
                                ps[0:mm, :],
                                lhsT=at[ap_][k2][:, m0:m0 + mm],
                                rhs=hslice(k2, hw),
                                start=(nmm == 0), stop=(nmm == 5))
                            nmm += 1
                    nc.vector.tensor_copy(e_sb[p][m][:, :], ps[0:mm, :])
                    dram = ere if p == "re" else eim
                    eng = nc.sync if p == "re" else nc.scalar
                    eng.dma_start(dram[m0:m0 + mm, :], e_sb[p][m][:, :])

    nc.finalize()
    return nc


_NC_CACHE = None


def _get_nc():
    global _NC_CACHE
    if _NC_CACHE is None:
        _NC_CACHE = _build_nc()
    return _NC_CACHE


def kernel(**inputs):
    global LAST_EXEC_TIME_NS
    inputs = {k: np.asarray(v) for k, v in inputs.items()}
    consts = _host_consts()
    in_maps = [_host_prep_sample(b, inputs, consts) for b in range(B)]
    nc = _get_nc()
    trace = bool(os.environ.get("BASS_TRACE"))
    res = run_bass_kernel_spmd(nc, in_maps, list(range(B)), trace=trace)
    LAST_EXEC_TIME_NS = res.exec_time_ns
    out = np.zeros((B, 1, N1, N1), np.complex64)
    for b in range(B):
        out[b, 0] = (res.results[b]["ere"].astype(np.float32)
                     + 1j * res.results[b]["eim"].astype(np.float32))
    return out


# revision 20
# speedup vs baseline: 1.1174x; 1.1174x over previous
"""FNS spectral network kernel v7 for 8x TRN2 NeuronCores (1 sample/core).

Math per sample b (validated vs reference in fp64 numpy, rel err ~3e-7):
    rh = (-Gi) @ r @ Gi.T ; x = conv1..conv3 -> *theta -> conv4..conv6 ;
    e  = H @ x @ H.T

v7 = v2 device mechanics (proven on HW) + scheduling fixes:
  - const loads reordered: front-critical (r16, g's, t1) on sync; all bulk
    tensors (t2..t6 via tcat, hcat, thet) on the gpsimd SWDGE ring so the
    HWDGE queues stay free for the x1 scatter / halo exchange.
  - thet fully SBUF-resident (prefetched at t=0) -- no conv3 DMA stalls.
  - xoc scatter DMAs interleaved into the conv6 loop (hidden behind
    compute) instead of one serial burst afterwards.
  - fp16 outputs; output DMAs split sync/scalar and interleaved with the
    e-stage so the tail is short.
"""

import os

import numpy as np

import concourse.bacc as bacc
import concourse.mybir as mybir
from concourse.bass_utils import run_bass_kernel_spmd
from concourse.tile import TileContext

F16 = mybir.dt.float16
F32 = mybir.dt.float32

B = 8
N1 = 255
CROP = 257
CH = 8
NBLK = 43
BSTR = 260
XW = NBLK * BSTR
CHUNKS = [(0, 11), (11, 11), (22, 11), (33, 10)]   # (start, count)
THW = 2 * CROP                                     # theta cols per block
TKEYS = ["t2", "t2z", "t2b", "t3", "t3z", "t3b", "t3s", "t3sz", "t3sb",
         "t4", "t4z", "t4b", "t5", "t5z", "t5b"]
TOFF = {k: i * 384 for i, k in enumerate(TKEYS)}

LAST_EXEC_TIME_NS = None


# ----------------------------------------------------------------------------
# Host-side prep
# ----------------------------------------------------------------------------

def _host_consts():
    j = np.arange(CROP)[:, None]
    n = np.arange(N1)[None, :]
    Gi = (np.sin(np.pi * (j - 128) * (n + 1) / 256.0) / 256.0).astype(np.float32)
    k = np.arange(N1)[:, None]
    jj = np.arange(CROP)[None, :]
    H = np.exp(-2j * np.pi * k * (jj - 127.0) / 513.0)
    g1t = np.ascontiguousarray((-Gi).T.astype(np.float16))   # [255,257]
    g2t = np.ascontiguousarray(Gi.T.astype(np.float16))      # [255,257]
    hrt = np.ascontiguousarray(H.real.T.astype(np.float16))  # [257,255]
    hit = np.ascontiguousarray(H.imag.T.astype(np.float16))
    hnit = np.ascontiguousarray((-H.imag).T.astype(np.float16))
    return {
        "gcat": np.ascontiguousarray(np.concatenate([g1t, g2t], axis=1)),
        "hcat": np.ascontiguousarray(np.concatenate([hrt, hit, hnit], axis=1)),
    }


def _expand_w(wre, wim):
    Co, Ci = wre.shape[0], wre.shape[1]
    W = np.zeros((2 * Co, 2 * Ci, 3, 3), np.float32)
    W[:Co, :Ci] = wre
    W[:Co, Ci:] = -wim
    W[Co:, :Ci] = wim
    W[Co:, Ci:] = wre
    return W


def _wT(wre, wim):
    return (np.swapaxes(np.swapaxes(wre, 0, 1), -2, -1),
            -np.swapaxes(np.swapaxes(wim, 0, 1), -2, -1))


def _row_std(p):
    if p < 96:
        return 1 + p // 16, p % 16
    if p < 112:
        return 0, p - 96
    return 7, p - 112


def _col_std_dup(m):
    if m < 96:
        return m // 16, m % 16
    if m < 112:
        return 5, m - 96
    return 0, m - 112


def _col_c6(m):
    return m % 6, m // 6


def _build_T(Wexp, rowmap, colmap, K, M, zero_hi=False, zero_lo=False):
    T = np.zeros((K, 3 * M), np.float32)
    Cin2 = Wexp.shape[1]
    for p in range(K):
        il, cp = rowmap(p)
        if cp >= Cin2:
            continue
        if zero_hi and il >= 6:
            continue
        if zero_lo and il == 0:
            continue
        for dj in range(3):
            for m in range(M):
                inn, op = colmap(m)
                di = il - inn
                if 0 <= di <= 2:
                    T[p, dj * M + m] = Wexp[op, cp, di, dj]
    return T.astype(np.float16)


def _host_prep_sample(bidx, inputs, consts):
    s = {}
    s["r16"] = np.ascontiguousarray(inputs["r"][bidx, 0].astype(np.float16))
    s.update(consts)

    w1 = (inputs["w1_re"][bidx], inputs["w1_im"][bidx])
    w2 = (inputs["w2_re"][bidx], inputs["w2_im"][bidx])
    w3 = (inputs["w3_re"][bidx], inputs["w3_im"][bidx])

    W1r = _expand_w(*w1)[:, 0:1]
    W2 = _expand_w(*w2)
    W3 = _expand_w(*w3)
    W3s = np.concatenate([W3[CH:], W3[:CH]], axis=0)
    W4 = _expand_w(*_wT(*w3))
    W5 = _expand_w(*_wT(*w2))
    W6 = _expand_w(*_wT(*w1))

    def row_x1(p):
        return p, 0

    s["t1cat"] = np.ascontiguousarray(np.concatenate([
        _build_T(W1r, row_x1, _col_std_dup, 8, 128),
        _build_T(W1r, row_x1, _col_std_dup, 8, 128, zero_hi=True)], axis=1))

    tm = {}
    for key, W in (("t2", W2), ("t3", W3), ("t3s", W3s), ("t4", W4), ("t5", W5)):
        tm[key] = _build_T(W, _row_std, _col_std_dup, 128, 128)
        tm[key + "z"] = _build_T(W, _row_std, _col_std_dup, 128, 128, zero_lo=True)
        tm[key + "b"] = _build_T(W, _row_std, _col_std_dup, 128, 128, zero_hi=True)
    s["tcat"] = np.ascontiguousarray(
        np.concatenate([tm[k] for k in TKEYS], axis=1))
    s["t6cat"] = np.ascontiguousarray(np.concatenate([
        _build_T(W6, _row_std, _col_c6, 128, 12),
        _build_T(W6, _row_std, _col_c6, 128, 12, zero_lo=True),
        _build_T(W6, _row_std, _col_c6, 128, 12, zero_hi=True)], axis=1))

    # theta pack [128, NBLK*514]; sign baked: col0 block = +tr, col1 = -/+ti
    tr = inputs["theta_re"][bidx]
    ti = inputs["theta_im"][bidx]
    th = np.zeros((128, NBLK * THW), np.float16)
    for b in range(NBLK):
        base = b * THW
        ninn = 6 if b < NBLK - 1 else 5
        for p in range(128):
            if p < 96:
                inn, op = p // 16, p % 16
            elif p < 112:
                inn, op = 5, p - 96
            else:
                inn, op = 0, p - 112
            if inn >= ninn:
                continue
            row = 6 * b + inn
            ch = op % 8
            th[p, base:base + CROP] = tr[ch, row]
            th[p, base + CROP:base + THW] = (-ti[ch, row]) if op < 8 else ti[ch, row]
    s["thet"] = th
    return s


# ----------------------------------------------------------------------------
# Device program
# ----------------------------------------------------------------------------

def _build_nc():
    nc = bacc.Bacc(None, target_bir_lowering=False, debug=False)

    dp = {}
    decls = [("r16", [N1, N1]), ("gcat", [N1, 2 * CROP]),
             ("hcat", [CROP, 3 * N1]), ("t1cat", [8, 768]),
             ("tcat", [128, 15 * 384]), ("t6cat", [128, 108]),
             ("thet", [128, NBLK * THW])]
    for name, shape in decls:
        dp[name] = nc.declare_dram_parameter(name, list(shape), F16,
                                             isOutput=False)
    ere = nc.declare_dram_parameter("ere", [N1, N1], F16, isOutput=True)
    eim = nc.declare_dram_parameter("eim", [N1, N1], F16, isOutput=True)

    with TileContext(nc) as tc:
        with (
            tc.tile_pool(name="const", bufs=1) as pc,
            tc.tile_pool(name="xbuf", bufs=1) as px,
            tc.tile_pool(name="work", bufs=1) as pw,
            tc.tile_pool(name="wk2", bufs=3) as pw2,
            tc.tile_pool(name="psum", bufs=8, space="PSUM") as pp,
        ):
            # ---------------- constant loads ----------------
            # front-critical on sync (HWDGE); bulk on gpsimd (SWDGE)
            r_sb = [pc.tile([128, N1], F16, name="r0", tag="r0"),
                    pc.tile([127, N1], F16, name="r1", tag="r1")]
            nc.sync.dma_start(r_sb[0][:, :], dp["r16"][0:128, :])
            nc.sync.dma_start(r_sb[1][:, :], dp["r16"][128:255, :])
            g_sb = [pc.tile([128, 2 * CROP], F16, name="g0", tag="g0"),
                    pc.tile([127, 2 * CROP], F16, name="g1", tag="g1")]
            nc.sync.dma_start(g_sb[0][:, :], dp["gcat"][0:128, :])
            nc.sync.dma_start(g_sb[1][:, :], dp["gcat"][128:255, :])
            t1c = pc.tile([8, 768], F16, name="t1c", tag="t1c")
            nc.sync.dma_start(t1c[:, :], dp["t1cat"][:, :])

            tcat = pc.tile([128, 15 * 384], F16, name="tcat", tag="tcat")
            nc.gpsimd.dma_start(tcat[:, :], dp["tcat"][:, :])
            t6c = pc.tile([128, 108], F16, name="t6c", tag="t6c")
            nc.gpsimd.dma_start(t6c[:, :], dp["t6cat"][:, :])
            thet = pc.tile([128, NBLK * THW], F16, name="thet", tag="thet")
            half = (NBLK // 2) * THW
            nc.gpsimd.dma_start(thet[:, 0:half], dp["thet"][:, 0:half])
            nc.gpsimd.dma_start(thet[:, half:], dp["thet"][:, half:])
            h_sb = [pc.tile([128, 3 * N1], F16, name="h0", tag="h0"),
                    pc.tile([128, 3 * N1], F16, name="h1", tag="h1"),
                    pc.tile([1, 3 * N1], F16, name="h2", tag="h2")]
            nc.gpsimd.dma_start(h_sb[0][:, :], dp["hcat"][0:128, :])
            nc.gpsimd.dma_start(h_sb[1][:, :], dp["hcat"][128:256, :])
            nc.gpsimd.dma_start(h_sb[2][:, :], dp["hcat"][256:257, :])

            def hslice(k2, which):
                return h_sb[k2][:, which * N1:(which + 1) * N1]

            def tvar(key, b):
                if b == 0 and key + "z" in TOFF:
                    key = key + "z"
                elif b == NBLK - 1 and key + "b" in TOFF:
                    key = key + "b"
                return TOFF[key]

            def tls(key, b, dj):
                off = tvar(key, b)
                return tcat[:, off + dj * 128:off + (dj + 1) * 128]

            def t1ls(b, dj):
                off = 384 if b == NBLK - 1 else 0
                return t1c[:, off + dj * 128:off + (dj + 1) * 128]

            # ---------------- big X tiles + pads ----------------
            x1 = px.tile([8, XW], F16, name="x1", tag="x1")
            XA = px.tile([128, XW], F16, name="XA", tag="XA")
            XB = px.tile([128, XW], F16, name="XB", tag="XB")
            XC = px.tile([128, XW], F16, name="XC", tag="XC")
            y6 = px.tile([12, NBLK * CROP], F16, name="y6", tag="y6")

            for X in (x1, XA, XB, XC):
                v = X[:, :].rearrange("p (b c) -> p b c", c=BSTR)
                nc.vector.memset(v[:, :, 0:1], 0.0)
                nc.vector.memset(v[:, :, 258:260], 0.0)
            nc.vector.memset(x1[0:1, 0:BSTR], 0.0)
            nc.vector.memset(x1[0:8, BSTR * 42:], 0.0)
            for X in (XA, XB, XC):
                nc.vector.memset(X[96:112, 0:BSTR], 0.0)
                nc.vector.memset(X[96:128, BSTR * 42:], 0.0)

            # ---------------- front transform ----------------
            vt_sb = [pw.tile([128, CROP], F16, name="vt0", tag="vt0"),
                     pw.tile([127, CROP], F16, name="vt1", tag="vt1")]
            for m, (m0, mm) in enumerate(((0, 128), (128, 127))):
                ps = pp.tile([128, CROP], F32, name="ps", tag="ps", bufs=3)
                for k2 in range(2):
                    nc.tensor.matmul(
                        ps[0:mm, :], lhsT=r_sb[k2][:, m0:m0 + mm],
                        rhs=g_sb[k2][:, 0:CROP], start=(k2 == 0), stop=(k2 == 1))
                nc.scalar.copy(vt_sb[m][:, :], ps[0:mm, :])

            rh_sb = [pw.tile([128, CROP], F16, name="rh0", tag="rh0"),
                     pw.tile([128, CROP], F16, name="rh1", tag="rh1"),
                     pw.tile([1, CROP], F16, name="rh2", tag="rh2")]
            for m, (m0, mm) in enumerate(((0, 128), (128, 128), (256, 1))):
                ps = pp.tile([128, CROP], F32, name="ps", tag="ps", bufs=3)
                for k2 in range(2):
                    nc.tensor.matmul(
                        ps[0:mm, :], lhsT=vt_sb[k2][:, m0:m0 + mm],
                        rhs=g_sb[k2][:, CROP:2 * CROP],
                        start=(k2 == 0), stop=(k2 == 1))
                nc.vector.tensor_copy(rh_sb[m][:, :], ps[0:mm, :])

            # x1 scatter: rows 6b-1..6b+6 -> x1[0:8, block b window], per
            # block 1-2 contiguous-partition DMAs (v2-proven plain APs).
            for b in range(NBLK):
                lo = max(0, 6 * b - 1)
                hi = min(256, 6 * b + 6)
                r0 = lo
                while r0 <= hi:
                    c = r0 // 128
                    c_end = min(hi, c * 128 + 127)
                    cnt = c_end - r0 + 1
                    il0 = r0 - (6 * b - 1)
                    (nc.sync if b % 2 else nc.scalar).dma_start(
                        x1[il0:il0 + cnt, BSTR * b + 1:BSTR * b + 258],
                        rh_sb[c][r0 - c * 128:r0 - c * 128 + cnt, :])
                    r0 = c_end + 1

            # ---------------- conv machinery ----------------
            S_W = 11 * CROP

            def strips_stage(XO, cstart, cnt, Svar):
                xv = XO[:, :].rearrange("p (b c) -> p b c", c=BSTR)
                sv = Svar[:, :].rearrange("p (b c) -> p b c", c=CROP)
                nc.sync.dma_start(sv[:, 0:cnt, :],
                                  xv[96:128, cstart:cstart + cnt, 1:258])

            def strips_fill_main(XO, cstart, cnt, Svar, eng_r):
                xv = XO[:, :].rearrange("p (b c) -> p b c", c=BSTR)
                sv = Svar[:, :].rearrange("p (b c) -> p b c", c=CROP)
                nb2 = cnt - 1
                if nb2 > 0:
                    eng_r.dma_start(
                        xv[96:112, cstart + 1:cstart + 1 + nb2, 1:258],
                        sv[0:16, 0:nb2, :])
                o = 1 if cstart == 0 else 0
                nb3 = cnt - o
                if nb3 > 0:
                    nc.scalar.dma_start(
                        xv[112:128, cstart + o - 1:cstart + o - 1 + nb3, 1:258],
                        sv[16:32, o:o + nb3, :])

            def strips_fill_cross(XO, cstart, cnt, Svar, eng_r):
                if cstart + cnt >= NBLK:
                    return
                xv = XO[:, :].rearrange("p (b c) -> p b c", c=BSTR)
                sv = Svar[:, :].rearrange("p (b c) -> p b c", c=CROP)
                eng_r.dma_start(
                    xv[96:112, cstart + cnt:cstart + cnt + 1, 1:258],
                    sv[0:16, cnt - 1:cnt, :])

            def conv_layer(tsel, XI, XO, kin, eng_r):
                prevS = None
                for ci, (cstart, cnt) in enumerate(CHUNKS):
                    for b in range(cstart, cstart + cnt):
                        ps = pp.tile([128, CROP], F32, name="ps", tag="ps",
                                     bufs=3)
                        for dj in range(3):
                            nc.tensor.matmul(
                                ps[:, :], lhsT=tsel(b, dj),
                                rhs=XI[0:kin, BSTR * b + dj:BSTR * b + dj + CROP],
                                start=(dj == 0), stop=(dj == 2))
                        dst = XO[:, BSTR * b + 1:BSTR * b + 1 + CROP]
                        if b % 2 == 0:
                            nc.vector.tensor_copy(dst, ps[:, :])
                        else:
                            nc.scalar.copy(dst, ps[:, :])
                    Svar = pw2.tile([32, S_W], F16, name="S", tag="S", bufs=3)
                    strips_stage(XO, cstart, cnt, Svar)
                    if prevS is not None:
                        strips_fill_cross(XO, *prevS, eng_r)
                    strips_fill_main(XO, cstart, cnt, Svar, eng_r)
                    prevS = (cstart, cnt, Svar)

            conv_layer(t1ls, x1, XA, 8, nc.sync)
            conv_layer(lambda b, dj: tls("t2", b, dj), XA, XB, 128, nc.sync)

            # conv3 + theta: XB -> XC (thet is SBUF-resident; no chunk DMAs)
            prevS = None
            for ci, (cstart, cnt) in enumerate(CHUNKS):
                for b in range(cstart, cstart + cnt):
                    tb = b * THW
                    psA = pp.tile([128, CROP], F32, name="psA", tag="psA",
                                  bufs=2)
                    psB = pp.tile([128, CROP], F32, name="psB", tag="psB",
                                  bufs=2)
                    for dj in range(3):
                        rhs = XB[:, BSTR * b + dj:BSTR * b + dj + CROP]
                        nc.tensor.matmul(
                            psA[:, :], lhsT=tls("t3", b, dj),
                            rhs=rhs, start=(dj == 0), stop=(dj == 2))
                    for dj in range(3):
                        rhs = XB[:, BSTR * b + dj:BSTR * b + dj + CROP]
                        nc.tensor.matmul(
                            psB[:, :], lhsT=tls("t3s", b, dj),
                            rhs=rhs, start=(dj == 0), stop=(dj == 2))
                    u = pw2.tile([128, CROP], F16, name="u", tag="u", bufs=4)
                    v = pw2.tile([128, CROP], F16, name="v", tag="v", bufs=4)
                    nc.vector.tensor_mul(u[:, :], psA[:, :],
                                         thet[:, tb:tb + CROP])
                    nc.vector.tensor_mul(v[:, :], psB[:, :],
                                         thet[:, tb + CROP:tb + THW])
                    nc.gpsimd.tensor_add(
                        XC[:, BSTR * b + 1:BSTR * b + 1 + CROP],
                        u[:, :], v[:, :])
                Svar = pw2.tile([32, S_W], F16, name="S", tag="S", bufs=3)
                strips_stage(XC, cstart, cnt, Svar)
                if prevS is not None:
                    strips_fill_cross(XC, *prevS, nc.sync)
                strips_fill_main(XC, cstart, cnt, Svar, nc.sync)
                prevS = (cstart, cnt, Svar)

            conv_layer(lambda b, dj: tls("t4", b, dj), XC, XA, 128, nc.gpsimd)
            conv_layer(lambda b, dj: tls("t5", b, dj), XA, XB, 128, nc.gpsimd)

            # conv6: XB -> y6, with the xoc scatter interleaved per block
            xoc = [pw.tile([128, 2 * CROP], F16, name="xoc0", tag="xoc0"),
                   pw.tile([128, 2 * CROP], F16, name="xoc1", tag="xoc1"),
                   pw.tile([1, 2 * CROP], F16, name="xoc2", tag="xoc2")]

            def scatter_block(b):
                pieces = []
                ninn = 6 if b < NBLK - 1 else 5
                i0 = 0
                while i0 < ninn:
                    r = 6 * b + i0
                    c = r // 128
                    csz = 128 if c < 2 else 1
                    iend = min(ninn - 1, (c * 128 + csz - 1 - 6 * b))
                    pieces.append((c, i0, iend - i0 + 1))
                    i0 = iend + 1
                for (c, i0, ni) in pieces:
                    p0 = 6 * b + i0 - 128 * c
                    for op in range(2):
                        eng = (nc.sync, nc.scalar, nc.gpsimd)[(2 * b + op) % 3]
                        eng.dma_start(
                            xoc[c][p0:p0 + ni, op * CROP:op * CROP + CROP],
                            y6[6 * op + i0:6 * op + i0 + ni,
                               CROP * b:CROP * (b + 1)])

            for b in range(NBLK):
                off = 36 if b == 0 else (72 if b == NBLK - 1 else 0)
                ps = pp.tile([128, CROP], F32, name="ps", tag="ps", bufs=3)
                for dj in range(3):
                    nc.tensor.matmul(
                        ps[0:12, :], lhsT=t6c[:, off + dj * 12:off + dj * 12 + 12],
                        rhs=XB[:, BSTR * b + dj:BSTR * b + dj + CROP],
                        start=(dj == 0), stop=(dj == 2))
                dst = y6[:, CROP * b:CROP * (b + 1)]
                if b % 2 == 0:
                    nc.vector.tensor_copy(dst, ps[0:12, :])
                else:
                    nc.scalar.copy(dst, ps[0:12, :])
                if b >= 2:
                    scatter_block(b - 2)
            scatter_block(NBLK - 2)
            scatter_block(NBLK - 1)

            # ---------------- back transform ----------------
            at = {}
            for p in ("re", "im"):
                at[p] = [pw.tile([128, N1], F16, name=f"at{p}0", tag=f"at{p}0"),
                         pw.tile([128, N1], F16, name=f"at{p}1", tag=f"at{p}1"),
                         pw.tile([1, N1], F16, name=f"at{p}2", tag=f"at{p}2")]
            for m, (m0, mm) in enumerate(((0, 128), (128, 128), (256, 1))):
                for p, terms in (("re", ((0, 0), (1, 2))),
                                 ("im", ((0, 1), (1, 0)))):
                    ps = pp.tile([128, N1], F32, name="ps", tag="ps", bufs=3)
                    nmm = 0
                    for (xi, hw) in terms:
                        for k2 in range(3):
                            nc.tensor.matmul(
                                ps[0:mm, :],
                                lhsT=xoc[k2][:, xi * CROP + m0:
                                             xi * CROP + m0 + mm],
                                rhs=hslice(k2, hw),
                                start=(nmm == 0), stop=(nmm == 5))
                            nmm += 1
                    nc.scalar.copy(at[p][m][:, :], ps[0:mm, :])

            e_sb = {}
            for p in ("re", "im"):
                e_sb[p] = [pw.tile([128, N1], F16, name=f"e{p}0", tag=f"e{p}0"),
                           pw.tile([127, N1], F16, name=f"e{p}1", tag=f"e{p}1")]
            for m, (m0, mm) in enumerate(((0, 128), (128, 127))):
                for p, terms in (("re", (("re", 0), ("im", 2))),
                                 ("im", (("re", 1), ("im", 0)))):
                    ps = pp.tile([128, N1], F32, name="ps", tag="ps", bufs=3)
                    nmm = 0
                    for (ap_, hw) in terms:
                        for k2 in range(3):
                            nc.tensor.matmul(
                                ps[0:mm, :],
                                lhsT=at[ap_][k2][:, m0:m0 + mm],
                                rhs=hslice(k2, hw),
                                start=(nmm == 0), stop=(nmm == 5))
                            nmm += 1
                    nc.vector.tensor_copy(e_sb[p][m][:, :], ps[0:mm, :])
                    dram = ere if p == "re" else eim
                    eng = nc.sync if p == "re" else nc.scalar
                    eng.dma_start(dram[m0:m0 + mm, :], e_sb[p][m][:, :])

    nc.finalize()
    return nc


_NC_CACHE = None


def _get_nc():
    global _NC_CACHE
    if _NC_CACHE is None:
        _NC_CACHE = _build_nc()
    return _NC_CACHE


def kernel(**inputs):
    global LAST_EXEC_TIME_NS
    inputs = {k: np.asarray(v) for k, v in inputs.items()}
    consts = _host_consts()
    in_maps = [_host_prep_sample(b, inputs, consts) for b in range(B)]
    nc = _get_nc()
    trace = bool(os.environ.get("BASS_TRACE"))
    res = run_bass_kernel_spmd(nc, in_maps, list(range(B)), trace=trace)
    LAST_EXEC_TIME_NS = res.exec_time_ns
    out = np.zeros((B, 1, N1, N1), np.complex64)
    for b in range(B):
        out[b, 0] = (res.results[b]["ere"].astype(np.float32)
                     + 1j * res.results[b]["eim"].astype(np.float32))
    return out


# revision 21
# speedup vs baseline: 1.1406x; 1.0208x over previous
"""FNS spectral network kernel v7 for 8x TRN2 NeuronCores (1 sample/core).

Math per sample b (validated vs reference in fp64 numpy, rel err ~3e-7):
    rh = (-Gi) @ r @ Gi.T ; x = conv1..conv3 -> *theta -> conv4..conv6 ;
    e  = H @ x @ H.T

v7 = v2 device mechanics (proven on HW) + scheduling fixes:
  - const loads reordered: front-critical (r16, g's, t1) on sync; all bulk
    tensors (t2..t6 via tcat, hcat, thet) on the gpsimd SWDGE ring so the
    HWDGE queues stay free for the x1 scatter / halo exchange.
  - thet fully SBUF-resident (prefetched at t=0) -- no conv3 DMA stalls.
  - xoc scatter DMAs interleaved into the conv6 loop (hidden behind
    compute) instead of one serial burst afterwards.
  - fp16 outputs; output DMAs split sync/scalar and interleaved with the
    e-stage so the tail is short.
"""

import os

import numpy as np

import concourse.bacc as bacc
import concourse.mybir as mybir
from concourse.bass_utils import run_bass_kernel_spmd
from concourse.tile import TileContext

F16 = mybir.dt.float16
F32 = mybir.dt.float32

B = 8
N1 = 255
CROP = 257
CH = 8
NBLK = 43
BSTR = 260
XW = NBLK * BSTR
CHUNKS = [(0, 11), (11, 11), (22, 11), (33, 10)]   # (start, count)
THW = 2 * CROP                                     # theta cols per block
TKEYS = ["t2", "t2z", "t2b", "t3", "t3z", "t3b", "t3s", "t3sz", "t3sb",
         "t4", "t4z", "t4b", "t5", "t5z", "t5b"]
TOFF = {k: i * 384 for i, k in enumerate(TKEYS)}

LAST_EXEC_TIME_NS = None


# ----------------------------------------------------------------------------
# Host-side prep
# ----------------------------------------------------------------------------

def _host_consts():
    j = np.arange(CROP)[:, None]
    n = np.arange(N1)[None, :]
    Gi = (np.sin(np.pi * (j - 128) * (n + 1) / 256.0) / 256.0).astype(np.float32)
    k = np.arange(N1)[:, None]
    jj = np.arange(CROP)[None, :]
    H = np.exp(-2j * np.pi * k * (jj - 127.0) / 513.0)
    g1t = np.ascontiguousarray((-Gi).T.astype(np.float16))   # [255,257]
    g2t = np.ascontiguousarray(Gi.T.astype(np.float16))      # [255,257]
    hrt = np.ascontiguousarray(H.real.T.astype(np.float16))  # [257,255]
    hit = np.ascontiguousarray(H.imag.T.astype(np.float16))
    hnit = np.ascontiguousarray((-H.imag).T.astype(np.float16))
    return {
        "gcat": np.ascontiguousarray(np.concatenate([g1t, g2t], axis=1)),
        "hcat": np.ascontiguousarray(np.concatenate([hrt, hit, hnit], axis=1)),
    }


def _expand_w(wre, wim):
    Co, Ci = wre.shape[0], wre.shape[1]
    W = np.zeros((2 * Co, 2 * Ci, 3, 3), np.float32)
    W[:Co, :Ci] = wre
    W[:Co, Ci:] = -wim
    W[Co:, :Ci] = wim
    W[Co:, Ci:] = wre
    return W


def _wT(wre, wim):
    return (np.swapaxes(np.swapaxes(wre, 0, 1), -2, -1),
            -np.swapaxes(np.swapaxes(wim, 0, 1), -2, -1))


def _row_std(p):
    if p < 96:
        return 1 + p // 16, p % 16
    if p < 112:
        return 0, p - 96
    return 7, p - 112


def _col_std_dup(m):
    if m < 96:
        return m // 16, m % 16
    if m < 112:
        return 5, m - 96
    return 0, m - 112


def _col_c6(m):
    return m % 6, m // 6


def _build_T(Wexp, rowmap, colmap, K, M, zero_hi=False, zero_lo=False):
    T = np.zeros((K, 3 * M), np.float32)
    Cin2 = Wexp.shape[1]
    for p in range(K):
        il, cp = rowmap(p)
        if cp >= Cin2:
            continue
        if zero_hi and il >= 6:
            continue
        if zero_lo and il == 0:
            continue
        for dj in range(3):
            for m in range(M):
                inn, op = colmap(m)
                di = il - inn
                if 0 <= di <= 2:
                    T[p, dj * M + m] = Wexp[op, cp, di, dj]
    return T.astype(np.float16)


def _host_prep_sample(bidx, inputs, consts):
    s = {}
    s["r16"] = np.ascontiguousarray(inputs["r"][bidx, 0].astype(np.float16))
    s.update(consts)

    w1 = (inputs["w1_re"][bidx], inputs["w1_im"][bidx])
    w2 = (inputs["w2_re"][bidx], inputs["w2_im"][bidx])
    w3 = (inputs["w3_re"][bidx], inputs["w3_im"][bidx])

    W1r = _expand_w(*w1)[:, 0:1]
    W2 = _expand_w(*w2)
    W3 = _expand_w(*w3)
    W3s = np.concatenate([W3[CH:], W3[:CH]], axis=0)
    W4 = _expand_w(*_wT(*w3))
    W5 = _expand_w(*_wT(*w2))
    W6 = _expand_w(*_wT(*w1))

    def row_x1(p):
        return p, 0

    s["t1cat"] = np.ascontiguousarray(np.concatenate([
        _build_T(W1r, row_x1, _col_std_dup, 8, 128),
        _build_T(W1r, row_x1, _col_std_dup, 8, 128, zero_hi=True)], axis=1))

    tm = {}
    for key, W in (("t2", W2), ("t3", W3), ("t3s", W3s), ("t4", W4), ("t5", W5)):
        tm[key] = _build_T(W, _row_std, _col_std_dup, 128, 128)
        tm[key + "z"] = _build_T(W, _row_std, _col_std_dup, 128, 128, zero_lo=True)
        tm[key + "b"] = _build_T(W, _row_std, _col_std_dup, 128, 128, zero_hi=True)
    s["tcat"] = np.ascontiguousarray(
        np.concatenate([tm[k] for k in TKEYS], axis=1))
    s["t6cat"] = np.ascontiguousarray(np.concatenate([
        _build_T(W6, _row_std, _col_c6, 128, 12),
        _build_T(W6, _row_std, _col_c6, 128, 12, zero_lo=True),
        _build_T(W6, _row_std, _col_c6, 128, 12, zero_hi=True)], axis=1))

    # theta pack [128, NBLK*514]; sign baked: col0 block = +tr, col1 = -/+ti
    tr = inputs["theta_re"][bidx]
    ti = inputs["theta_im"][bidx]
    th = np.zeros((128, NBLK * THW), np.float16)
    for b in range(NBLK):
        base = b * THW
        ninn = 6 if b < NBLK - 1 else 5
        for p in range(128):
            if p < 96:
                inn, op = p // 16, p % 16
            elif p < 112:
                inn, op = 5, p - 96
            else:
                inn, op = 0, p - 112
            if inn >= ninn:
                continue
            row = 6 * b + inn
            ch = op % 8
            th[p, base:base + CROP] = tr[ch, row]
            th[p, base + CROP:base + THW] = (-ti[ch, row]) if op < 8 else ti[ch, row]
    s["thet"] = th
    return s


# ----------------------------------------------------------------------------
# Device program
# ----------------------------------------------------------------------------

def _build_nc():
    nc = bacc.Bacc(None, target_bir_lowering=False, debug=False)

    dp = {}
    decls = [("r16", [N1, N1]), ("gcat", [N1, 2 * CROP]),
             ("hcat", [CROP, 3 * N1]), ("t1cat", [8, 768]),
             ("tcat", [128, 15 * 384]), ("t6cat", [128, 108]),
             ("thet", [128, NBLK * THW])]
    for name, shape in decls:
        dp[name] = nc.declare_dram_parameter(name, list(shape), F16,
                                             isOutput=False)
    ere = nc.declare_dram_parameter("ere", [N1, N1], F16, isOutput=True)
    eim = nc.declare_dram_parameter("eim", [N1, N1], F16, isOutput=True)

    with TileContext(nc) as tc:
        with (
            tc.tile_pool(name="const", bufs=1) as pc,
            tc.tile_pool(name="xbuf", bufs=1) as px,
            tc.tile_pool(name="work", bufs=1) as pw,
            tc.tile_pool(name="wk2", bufs=3) as pw2,
            tc.tile_pool(name="psum", bufs=8, space="PSUM") as pp,
        ):
            # ---------------- constant loads ----------------
            # front-critical on sync (HWDGE); bulk on gpsimd (SWDGE)
            r_sb = [pc.tile([128, N1], F16, name="r0", tag="r0"),
                    pc.tile([127, N1], F16, name="r1", tag="r1")]
            nc.sync.dma_start(r_sb[0][:, :], dp["r16"][0:128, :])
            nc.sync.dma_start(r_sb[1][:, :], dp["r16"][128:255, :])
            g_sb = [pc.tile([128, 2 * CROP], F16, name="g0", tag="g0"),
                    pc.tile([127, 2 * CROP], F16, name="g1", tag="g1")]
            nc.sync.dma_start(g_sb[0][:, :], dp["gcat"][0:128, :])
            nc.sync.dma_start(g_sb[1][:, :], dp["gcat"][128:255, :])
            t1c = pc.tile([8, 768], F16, name="t1c", tag="t1c")
            nc.sync.dma_start(t1c[:, :], dp["t1cat"][:, :])

            # Bulk loads saturate HBM (~358 GB/s shared); make the gpsimd
            # queue wait for the front-critical bytes before issuing them.
            dep_sb = pc.tile([1, 8], F16, name="dep_sb", tag="dep_sb")
            nc.gpsimd.tensor_copy(dep_sb[:, :], g_sb[1][0:1, 0:8])
            tcat = pc.tile([128, 15 * 384], F16, name="tcat", tag="tcat")
            nc.gpsimd.dma_start(tcat[:, :], dp["tcat"][:, :])
            t6c = pc.tile([128, 108], F16, name="t6c", tag="t6c")
            nc.gpsimd.dma_start(t6c[:, :], dp["t6cat"][:, :])
            thet = pc.tile([128, NBLK * THW], F16, name="thet", tag="thet")
            half = (NBLK // 2) * THW
            nc.gpsimd.dma_start(thet[:, 0:half], dp["thet"][:, 0:half])
            nc.gpsimd.dma_start(thet[:, half:], dp["thet"][:, half:])
            h_sb = [pc.tile([128, 3 * N1], F16, name="h0", tag="h0"),
                    pc.tile([128, 3 * N1], F16, name="h1", tag="h1"),
                    pc.tile([1, 3 * N1], F16, name="h2", tag="h2")]
            nc.gpsimd.dma_start(h_sb[0][:, :], dp["hcat"][0:128, :])
            nc.gpsimd.dma_start(h_sb[1][:, :], dp["hcat"][128:256, :])
            nc.gpsimd.dma_start(h_sb[2][:, :], dp["hcat"][256:257, :])

            def hslice(k2, which):
                return h_sb[k2][:, which * N1:(which + 1) * N1]

            def tvar(key, b):
                if b == 0 and key + "z" in TOFF:
                    key = key + "z"
                elif b == NBLK - 1 and key + "b" in TOFF:
                    key = key + "b"
                return TOFF[key]

            def tls(key, b, dj):
                off = tvar(key, b)
                return tcat[:, off + dj * 128:off + (dj + 1) * 128]

            def t1ls(b, dj):
                off = 384 if b == NBLK - 1 else 0
                return t1c[:, off + dj * 128:off + (dj + 1) * 128]

            # ---------------- big X tiles + pads ----------------
            x1 = px.tile([8, XW], F16, name="x1", tag="x1")
            XA = px.tile([128, XW], F16, name="XA", tag="XA")
            XB = px.tile([128, XW], F16, name="XB", tag="XB")
            XC = px.tile([128, XW], F16, name="XC", tag="XC")
            y6 = px.tile([12, NBLK * CROP], F16, name="y6", tag="y6")

            for X in (x1, XA, XB, XC):
                v = X[:, :].rearrange("p (b c) -> p b c", c=BSTR)
                nc.vector.memset(v[:, :, 0:1], 0.0)
                nc.vector.memset(v[:, :, 258:260], 0.0)
            nc.vector.memset(x1[0:1, 0:BSTR], 0.0)
            nc.vector.memset(x1[0:8, BSTR * 42:], 0.0)
            for X in (XA, XB, XC):
                nc.vector.memset(X[96:112, 0:BSTR], 0.0)
                nc.vector.memset(X[96:128, BSTR * 42:], 0.0)

            # ---------------- front transform ----------------
            vt_sb = [pw.tile([128, CROP], F16, name="vt0", tag="vt0"),
                     pw.tile([127, CROP], F16, name="vt1", tag="vt1")]
            for m, (m0, mm) in enumerate(((0, 128), (128, 127))):
                ps = pp.tile([128, CROP], F32, name="ps", tag="ps", bufs=3)
                for k2 in range(2):
                    nc.tensor.matmul(
                        ps[0:mm, :], lhsT=r_sb[k2][:, m0:m0 + mm],
                        rhs=g_sb[k2][:, 0:CROP], start=(k2 == 0), stop=(k2 == 1))
                nc.scalar.copy(vt_sb[m][:, :], ps[0:mm, :])

            rh_sb = [pw.tile([128, CROP], F16, name="rh0", tag="rh0"),
                     pw.tile([128, CROP], F16, name="rh1", tag="rh1"),
                     pw.tile([1, CROP], F16, name="rh2", tag="rh2")]
            for m, (m0, mm) in enumerate(((0, 128), (128, 128), (256, 1))):
                ps = pp.tile([128, CROP], F32, name="ps", tag="ps", bufs=3)
                for k2 in range(2):
                    nc.tensor.matmul(
                        ps[0:mm, :], lhsT=vt_sb[k2][:, m0:m0 + mm],
                        rhs=g_sb[k2][:, CROP:2 * CROP],
                        start=(k2 == 0), stop=(k2 == 1))
                nc.vector.tensor_copy(rh_sb[m][:, :], ps[0:mm, :])

            # x1 scatter: rows 6b-1..6b+6 -> x1[0:8, block b window], per
            # block 1-2 contiguous-partition DMAs (v2-proven plain APs).
            for b in range(NBLK):
                lo = max(0, 6 * b - 1)
                hi = min(256, 6 * b + 6)
                r0 = lo
                while r0 <= hi:
                    c = r0 // 128
                    c_end = min(hi, c * 128 + 127)
                    cnt = c_end - r0 + 1
                    il0 = r0 - (6 * b - 1)
                    (nc.sync if b % 2 else nc.scalar).dma_start(
                        x1[il0:il0 + cnt, BSTR * b + 1:BSTR * b + 258],
                        rh_sb[c][r0 - c * 128:r0 - c * 128 + cnt, :])
                    r0 = c_end + 1

            # ---------------- conv machinery ----------------
            S_W = 11 * CROP

            def strips_stage(XO, cstart, cnt, Svar):
                xv = XO[:, :].rearrange("p (b c) -> p b c", c=BSTR)
                sv = Svar[:, :].rearrange("p (b c) -> p b c", c=CROP)
                nc.sync.dma_start(sv[:, 0:cnt, :],
                                  xv[96:128, cstart:cstart + cnt, 1:258])

            def strips_fill_main(XO, cstart, cnt, Svar, eng_r):
                xv = XO[:, :].rearrange("p (b c) -> p b c", c=BSTR)
                sv = Svar[:, :].rearrange("p (b c) -> p b c", c=CROP)
                nb2 = cnt - 1
                if nb2 > 0:
                    eng_r.dma_start(
                        xv[96:112, cstart + 1:cstart + 1 + nb2, 1:258],
                        sv[0:16, 0:nb2, :])
                o = 1 if cstart == 0 else 0
                nb3 = cnt - o
                if nb3 > 0:
                    nc.scalar.dma_start(
                        xv[112:128, cstart + o - 1:cstart + o - 1 + nb3, 1:258],
                        sv[16:32, o:o + nb3, :])

            def strips_fill_cross(XO, cstart, cnt, Svar, eng_r):
                if cstart + cnt >= NBLK:
                    return
                xv = XO[:, :].rearrange("p (b c) -> p b c", c=BSTR)
                sv = Svar[:, :].rearrange("p (b c) -> p b c", c=CROP)
                eng_r.dma_start(
                    xv[96:112, cstart + cnt:cstart + cnt + 1, 1:258],
                    sv[0:16, cnt - 1:cnt, :])

            def conv_layer(tsel, XI, XO, kin, eng_r):
                prevS = None
                for ci, (cstart, cnt) in enumerate(CHUNKS):
                    for b in range(cstart, cstart + cnt):
                        ps = pp.tile([128, CROP], F32, name="ps", tag="ps",
                                     bufs=3)
                        for dj in range(3):
                            nc.tensor.matmul(
                                ps[:, :], lhsT=tsel(b, dj),
                                rhs=XI[0:kin, BSTR * b + dj:BSTR * b + dj + CROP],
                                start=(dj == 0), stop=(dj == 2))
                        dst = XO[:, BSTR * b + 1:BSTR * b + 1 + CROP]
                        if b % 2 == 0:
                            nc.vector.tensor_copy(dst, ps[:, :])
                        else:
                            nc.scalar.copy(dst, ps[:, :])
                    Svar = pw2.tile([32, S_W], F16, name="S", tag="S", bufs=3)
                    strips_stage(XO, cstart, cnt, Svar)
                    if prevS is not None:
                        strips_fill_cross(XO, *prevS, eng_r)
                    strips_fill_main(XO, cstart, cnt, Svar, eng_r)
                    prevS = (cstart, cnt, Svar)

            conv_layer(t1ls, x1, XA, 8, nc.sync)
            conv_layer(lambda b, dj: tls("t2", b, dj), XA, XB, 128, nc.sync)

            # conv3 + theta: XB -> XC (thet is SBUF-resident; no chunk DMAs)
            prevS = None
            for ci, (cstart, cnt) in enumerate(CHUNKS):
                for b in range(cstart, cstart + cnt):
                    tb = b * THW
                    psA = pp.tile([128, CROP], F32, name="psA", tag="psA",
                                  bufs=2)
                    psB = pp.tile([128, CROP], F32, name="psB", tag="psB",
                                  bufs=2)
                    for dj in range(3):
                        rhs = XB[:, BSTR * b + dj:BSTR * b + dj + CROP]
                        nc.tensor.matmul(
                            psA[:, :], lhsT=tls("t3", b, dj),
                            rhs=rhs, start=(dj == 0), stop=(dj == 2))
                    for dj in range(3):
                        rhs = XB[:, BSTR * b + dj:BSTR * b + dj + CROP]
                        nc.tensor.matmul(
                            psB[:, :], lhsT=tls("t3s", b, dj),
                            rhs=rhs, start=(dj == 0), stop=(dj == 2))
                    u = pw2.tile([128, CROP], F16, name="u", tag="u", bufs=4)
                    v = pw2.tile([128, CROP], F16, name="v", tag="v", bufs=4)
                    nc.vector.tensor_mul(u[:, :], psA[:, :],
                                         thet[:, tb:tb + CROP])
                    nc.vector.tensor_mul(v[:, :], psB[:, :],
                                         thet[:, tb + CROP:tb + THW])
                    nc.gpsimd.tensor_add(
                        XC[:, BSTR * b + 1:BSTR * b + 1 + CROP],
                        u[:, :], v[:, :])
                Svar = pw2.tile([32, S_W], F16, name="S", tag="S", bufs=3)
                strips_stage(XC, cstart, cnt, Svar)
                if prevS is not None:
                    strips_fill_cross(XC, *prevS, nc.sync)
                strips_fill_main(XC, cstart, cnt, Svar, nc.sync)
                prevS = (cstart, cnt, Svar)

            conv_layer(lambda b, dj: tls("t4", b, dj), XC, XA, 128, nc.gpsimd)
            conv_layer(lambda b, dj: tls("t5", b, dj), XA, XB, 128, nc.gpsimd)

            # conv6: XB -> y6, with the xoc scatter interleaved per block
            xoc = [pw.tile([128, 2 * CROP], F16, name="xoc0", tag="xoc0"),
                   pw.tile([128, 2 * CROP], F16, name="xoc1", tag="xoc1"),
                   pw.tile([1, 2 * CROP], F16, name="xoc2", tag="xoc2")]

            def scatter_block(b):
                pieces = []
                ninn = 6 if b < NBLK - 1 else 5
                i0 = 0
                while i0 < ninn:
                    r = 6 * b + i0
                    c = r // 128
                    csz = 128 if c < 2 else 1
                    iend = min(ninn - 1, (c * 128 + csz - 1 - 6 * b))
                    pieces.append((c, i0, iend - i0 + 1))
                    i0 = iend + 1
                for (c, i0, ni) in pieces:
                    p0 = 6 * b + i0 - 128 * c
                    for op in range(2):
                        eng = (nc.sync, nc.scalar, nc.gpsimd)[(2 * b + op) % 3]
                        eng.dma_start(
                            xoc[c][p0:p0 + ni, op * CROP:op * CROP + CROP],
                            y6[6 * op + i0:6 * op + i0 + ni,
                               CROP * b:CROP * (b + 1)])

            for b in range(NBLK):
                off = 36 if b == 0 else (72 if b == NBLK - 1 else 0)
                ps = pp.tile([128, CROP], F32, name="ps", tag="ps", bufs=3)
                for dj in range(3):
                    nc.tensor.matmul(
                        ps[0:12, :], lhsT=t6c[:, off + dj * 12:off + dj * 12 + 12],
                        rhs=XB[:, BSTR * b + dj:BSTR * b + dj + CROP],
                        start=(dj == 0), stop=(dj == 2))
                dst = y6[:, CROP * b:CROP * (b + 1)]
                if b % 2 == 0:
                    nc.vector.tensor_copy(dst, ps[0:12, :])
                else:
                    nc.scalar.copy(dst, ps[0:12, :])
                if b >= 2:
                    scatter_block(b - 2)
            scatter_block(NBLK - 2)
            scatter_block(NBLK - 1)

            # ---------------- back transform ----------------
            at = {}
            for p in ("re", "im"):
                at[p] = [pw.tile([128, N1], F16, name=f"at{p}0", tag=f"at{p}0"),
                         pw.tile([128, N1], F16, name=f"at{p}1", tag=f"at{p}1"),
                         pw.tile([1, N1], F16, name=f"at{p}2", tag=f"at{p}2")]
            for m, (m0, mm) in enumerate(((0, 128), (128, 128), (256, 1))):
                for p, terms in (("re", ((0, 0), (1, 2))),
                                 ("im", ((0, 1), (1, 0)))):
                    ps = pp.tile([128, N1], F32, name="ps", tag="ps", bufs=3)
                    nmm = 0
                    for (xi, hw) in terms:
                        for k2 in range(3):
                            nc.tensor.matmul(
                                ps[0:mm, :],
                                lhsT=xoc[k2][:, xi * CROP + m0:
                                             xi * CROP + m0 + mm],
                                rhs=hslice(k2, hw),
                                start=(nmm == 0), stop=(nmm == 5))
                            nmm += 1
                    nc.scalar.copy(at[p][m][:, :], ps[0:mm, :])

            e_sb = {}
            for p in ("re", "im"):
                e_sb[p] = [pw.tile([128, N1], F16, name=f"e{p}0", tag=f"e{p}0"),
                           pw.tile([127, N1], F16, name=f"e{p}1", tag=f"e{p}1")]
            for m, (m0, mm) in enumerate(((0, 128), (128, 127))):
                for p, terms in (("re", (("re", 0), ("im", 2))),
                                 ("im", (("re", 1), ("im", 0)))):
                    ps = pp.tile([128, N1], F32, name="ps", tag="ps", bufs=3)
                    nmm = 0
                    for (ap_, hw) in terms:
                        for k2 in range(3):
                            nc.tensor.matmul(
                                ps[0:mm, :],
                                lhsT=at[ap_][k2][:, m0:m0 + mm],
                                rhs=hslice(k2, hw),
                                start=(nmm == 0), stop=(nmm == 5))
                            nmm += 1
                    nc.vector.tensor_copy(e_sb[p][m][:, :], ps[0:mm, :])
                    dram = ere if p == "re" else eim
                    eng = nc.sync if p == "re" else nc.scalar
                    eng.dma_start(dram[m0:m0 + mm, :], e_sb[p][m][:, :])

    nc.finalize()
    return nc


_NC_CACHE = None


def _get_nc():
    global _NC_CACHE
    if _NC_CACHE is None:
        _NC_CACHE = _build_nc()
    return _NC_CACHE


def kernel(**inputs):
    global LAST_EXEC_TIME_NS
    inputs = {k: np.asarray(v) for k, v in inputs.items()}
    consts = _host_consts()
    in_maps = [_host_prep_sample(b, inputs, consts) for b in range(B)]
    nc = _get_nc()
    trace = bool(os.environ.get("BASS_TRACE"))
    res = run_bass_kernel_spmd(nc, in_maps, list(range(B)), trace=trace)
    LAST_EXEC_TIME_NS = res.exec_time_ns
    out = np.zeros((B, 1, N1, N1), np.complex64)
    for b in range(B):
        out[b, 0] = (res.results[b]["ere"].astype(np.float32)
                     + 1j * res.results[b]["eim"].astype(np.float32))
    return out
